# revision 85
# baseline (speedup 1.0000x reference)
"""Distributed Trainium2 kernel for nn_LossMeanCov (vq_codebook).

Two SPMD Bass launches on 8 NeuronCores:

Stage 1 (data-parallel over N): each core takes 8192 points and computes
  score[i,k] = x_i . c_k - |c_k|^2/2  via one bf16 matmul per 128-point
  tile (fp32 PSUM), then argmax over the 512 clusters with the DVE
  max/max_index instructions reading PSUM directly.  Output: per-point
  cluster id (uint16).

Host glue: counting-sort of the 65536 cluster ids, greedy bin-packing of
  the 512 clusters onto 8 cores (64 slots each, balancing the number of
  128-row tiles), and a padded gather of x rows into a per-core tile
  stream (zero rows pad each cluster to a 128 multiple).

Stage 2 (cluster-parallel): each core computes, per 128-row tile, the
  augmented Gram matrix [x,1]^T [x,1] (bf16 matmul -> counts, sums and
  x x^T sums in one [65,65] result), transposes the Gram batch so tiles
  lie on partitions, segment-sums tiles into 64 cluster slots with a
  single selection matmul, and finishes the means/covs/filling losses
  with batched vector ops ([slot, feature] layout -> per-cluster scalars
  are per-partition scalars).  Output: 3 partial loss sums per core.

Everything data-dependent (packing, gather indices, target permutations)
lives on the host; the device programs are static.  If the data ever
exceeds the static budgets (tile budget per core) the kernel falls back
to a numpy path that is correct for arbitrary inputs.
"""

import numpy as np
import ml_dtypes

BF16 = ml_dtypes.bfloat16

N, D, K, NCORES = 65536, 64, 512, 8
DA = D + 1            # augmented feature dim (ones column)
PT = 128              # points per stage-1 tile / rows per Gram tile
NT1 = N // NCORES // PT   # stage-1 tiles per core (64)
NPC = N // NCORES         # points per core (8192)
T2 = 112              # stage-2 tile budget per core
SLOTS = K // NCORES   # clusters per core (64)
GB = 14               # Gram matrices packed per 2-bank PSUM batch
TB = 7                # transpose columns per PSUM batch
SEG = 512             # segment-matmul free chunk (one PSUM bank)
GRID = DA * DA        # 4225

_programs = {}
LAST_TIMINGS = {}
LAST_INPUTS = {}


# --------------------------------------------------------------------------
# device programs
# --------------------------------------------------------------------------

def _build_stage1(rounds=1, variant="psum"):
    import concourse.tile as tile
    import concourse.bacc as bacc
    import concourse.mybir as mybir
    from contextlib import ExitStack

    f16, f32, u16 = mybir.dt.bfloat16, mybir.dt.float32, mybir.dt.uint16
    AF = mybir.ActivationFunctionType

    nc = bacc.Bacc("TRN2", target_bir_lowering=False, debug=False,
                   num_devices=NCORES)
    pdt = u16 if variant in ("psum", "sbuf16") else f32
    xT = nc.dram_tensor("xT", [DA, NPC], f16, kind="ExternalInput").ap()
    caug = nc.dram_tensor("caug", [DA, K], f16, kind="ExternalInput").ap()
    # full [pt, tile, 8] index dump; host reads [:, :, 0] (contiguous DMA)
    pred = nc.dram_tensor("pred", [PT, NT1 * 8], pdt,
                          kind="ExternalOutput").ap()

    with tile.TileContext(nc) as tc, ExitStack() as ctx:
        const = ctx.enter_context(tc.tile_pool(name="const", bufs=1))
        psum = ctx.enter_context(tc.tile_pool(name="ps", bufs=8, space="PSUM"))
        mxp = ctx.enter_context(tc.tile_pool(name="mx", bufs=8))
        scp = ctx.enter_context(tc.tile_pool(name="sc", bufs=4))

        xT_sb = const.tile([DA, NPC], f16)
        nc.sync.dma_start(xT_sb[:], xT[:])
        caug_sb = const.tile([DA, K], f16)
        nc.sync.dma_start(caug_sb[:], caug[:])
        pred_sb = const.tile([PT, NT1 * 8], pdt)

        for _ in range(rounds):
            for t in range(NT1):
                ps = psum.tile([PT, K], f32)
                nc.tensor.matmul(ps[:], lhsT=xT_sb[:, t * PT:(t + 1) * PT],
                                 rhs=caug_sb[:], start=True, stop=True)
                m8 = mxp.tile([PT, 8], f32)
                if variant == "psum":
                    nc.vector.max(m8[:], ps[:])
                    nc.vector.max_index(pred_sb[:, t * 8:(t + 1) * 8],
                                        m8[:], ps[:])
                elif variant == "sbuf16":
                    s = scp.tile([PT, K], f16)
                    nc.scalar.activation(s[:], ps[:], AF.Copy)
                    nc.vector.max(m8[:], s[:])
                    nc.vector.max_index(pred_sb[:, t * 8:(t + 1) * 8],
                                        m8[:], s[:])
                elif variant == "maxonly":
                    nc.vector.max(m8[:], ps[:])
                    nc.vector.tensor_copy(
                        pred_sb[:, t * 8:(t + 1) * 8], m8[:])
                elif variant == "mmonly":
                    nc.scalar.activation(m8[:], ps[:, 0:8], AF.Copy)
                    nc.vector.tensor_copy(
                        pred_sb[:, t * 8:(t + 1) * 8], m8[:])

            nc.sync.dma_start(pred[:], pred_sb[:])
    nc.compile()
    return nc


def _build_stage2(rounds=1, phases="gtsf"):
    import concourse.tile as tile
    import concourse.bacc as bacc
    import concourse.mybir as mybir
    from concourse.masks import make_identity
    from contextlib import ExitStack

    f16, f32 = mybir.dt.bfloat16, mybir.dt.float32
    A = mybir.AluOpType
    AF = mybir.ActivationFunctionType

    nc = bacc.Bacc("TRN2", target_bir_lowering=False, debug=False,
                   num_devices=NCORES)
    # xg layout: [p, t, c] so each partition's stream is contiguous
    xg = nc.dram_tensor("xg", [PT, T2 * DA], f16, kind="ExternalInput").ap()
    sel = nc.dram_tensor("sel", [T2, SLOTS], f16, kind="ExternalInput").ap()
    tfull = nc.dram_tensor("tfull", [SLOTS, GRID], f32, kind="ExternalInput").ap()
    # madj = (1-z)*centers - means_target ; hscal = per-slot scalars
    madj = nc.dram_tensor("madj", [SLOTS, D], f32, kind="ExternalInput").ap()
    hscal = nc.dram_tensor("hscal", [SLOTS, 4], f32, kind="ExternalInput").ap()
    out = nc.dram_tensor("partial", [SLOTS, 4], f32, kind="ExternalOutput").ap()

    with tile.TileContext(nc) as tc, ExitStack() as ctx:
        const = ctx.enter_context(tc.tile_pool(name="const", bufs=1))
        psg = ctx.enter_context(tc.tile_pool(name="psg", bufs=4, space="PSUM"))
        pst = ctx.enter_context(tc.tile_pool(name="pst", bufs=2, space="PSUM"))
        pss = ctx.enter_context(tc.tile_pool(name="pss", bufs=2, space="PSUM"))
        tmp = ctx.enter_context(tc.tile_pool(name="tmp", bufs=2))

        xg_sb = const.tile([PT, T2 * DA], f16)
        nc.sync.dma_start(xg_sb[:], xg[:])
        sel_sb = const.tile([T2, SLOTS], f16)
        nc.sync.dma_start(sel_sb[:], sel[:])
        tf_sb = const.tile([SLOTS, GRID], f32)
        nc.sync.dma_start(tf_sb[:], tfull[:])
        madj_sb = const.tile([SLOTS, D], f32)
        nc.sync.dma_start(madj_sb[:], madj[:])
        hs_sb = const.tile([SLOTS, 4], f32)
        nc.sync.dma_start(hs_sb[:], hscal[:])

        ident = const.tile([T2, T2], f16)
        make_identity(nc, ident[:])

        gall = const.tile([DA, T2 * DA], f16)
        tg = const.tile([T2, GRID], f16)
        s2 = const.tile([SLOTS, GRID], f32)
        og = const.tile([SLOTS, GRID], f32)
        qg = const.tile([SLOTS, GRID], f32)

        # body emitted `rounds` times (bench variants reuse the same tiles)
        for _round in range(rounds):
            _stage2_body(nc, tc, tmp, psg, pst, pss, xg_sb, sel_sb, tf_sb,
                         madj_sb, hs_sb, ident, gall, tg, s2, og, qg,
                         out, phases)
    nc.compile()
    return nc


def _stage2_body(nc, tc, tmp, psg, pst, pss, xg_sb, sel_sb, tf_sb,
                 madj_sb, hs_sb, ident, gall, tg, s2, og, qg, out,
                 phases="gtsf"):
    import concourse.mybir as mybir

    f16, f32 = mybir.dt.bfloat16, mybir.dt.float32
    A = mybir.AluOpType
    AF = mybir.ActivationFunctionType

    if "g" in phases:
        # ---- per-tile augmented Grams, batched 7 per PSUM bank ----
        PER = 7
        nb = (T2 + PER - 1) // PER
        for b in range(nb):
            g0 = b * PER
            g1 = min(g0 + PER, T2)
            gp = psg.tile([DA, PER * DA], f32)
            for g in range(g0, g1):
                xt = xg_sb[:].rearrange("p (t c) -> p t c", c=DA)[:, g, :]
                nc.tensor.matmul(gp[:, (g - g0) * DA:(g - g0 + 1) * DA],
                                 lhsT=xt, rhs=xt, start=True, stop=True)
            nc.scalar.activation(gall[:, g0 * DA:g1 * DA],
                                 gp[:, :(g1 - g0) * DA], AF.Copy)

    if "t" in phases:
        # ---- transpose tile-Gram batch: tg[t, c*DA+a] = G_t[a, c] ----
        gall3 = gall[:].rearrange("p (t c) -> p t c", c=DA)
        DAP = DA + 1  # 66: keeps PSUM fp16 chunk offsets 4B-aligned
        ntb = (DA + TB - 1) // TB
        for b in range(ntb):
            c0 = b * TB
            c1 = min(c0 + TB, DA)
            tp = pst.tile([T2, TB * DAP], f16)
            for c in range(c0, c1):
                nc.tensor.transpose(tp[:, (c - c0) * DAP:(c - c0) * DAP + DA],
                                    gall3[:, :, c], ident[0:DA, 0:DA])
            src = tp[:].rearrange("p (c e) -> p c e", e=DAP)[:, :c1 - c0, :DA]
            dst = tg[:, c0 * DA:c1 * DA].rearrange("p (c e) -> p c e", e=DA)
            nc.scalar.activation(dst, src, AF.Copy)

    if "s" in phases:
        # ---- segment-sum tiles into slots: s2[j, f] = sum_t sel[t,j] tg[t,f]
        # last chunk (holds s1/counts) first so finalize overlaps the rest
        nseg = (GRID + SEG - 1) // SEG
        for c in [nseg - 1] + list(range(nseg - 1)):
            f0 = c * SEG
            f1 = min(f0 + SEG, GRID)
            sp = pss.tile([SLOTS, SEG], f32)
            nc.tensor.matmul(sp[:, :f1 - f0], lhsT=sel_sb[:],
                             rhs=tg[:, f0:f1], start=True, stop=True)
            nc.scalar.activation(s2[:, f0:f1], sp[:, :f1 - f0], AF.Copy)

    if "f" in phases:
        # ---- finalize (rn/rdw/rnrdw/ftneg precomputed on host) ----
        sub = phases[phases.index("f") + 1:] or "mcl"
        s1 = s2[:, D * DA:D * DA + DA]          # [SLOTS, DA]; col D holds n
        nvec = s2[:, D * DA + D:D * DA + D + 1]  # [SLOTS, 1]
        rn, rdw, rnrdw, ftneg = (hs_sb[:, i:i + 1] for i in range(4))

        red = tmp.tile([SLOTS, 4], f32)
        # separate accumulator tiles: ACT accum_out and DVE writes must not
        # land in the same SBUF row concurrently (HW write hazard)
        lm_t = tmp.tile([SLOTS, 1], f32, tag="lmt")
        lc_t = tmp.tile([SLOTS, 1], f32, tag="lct")
        lf, lm, lc = red[:, 0:1], lm_t[:], lc_t[:]

        if "m" in sub:
            # means loss: mean+adj = s1*rn + ((1-z)*centers - means_target)
            msel = tmp.tile([SLOTS, D], f32)
            nc.vector.tensor_scalar(msel[:], s1[:, 0:D], rn, None, op0=A.mult)
            nc.vector.tensor_tensor(out=msel[:], in0=msel[:], in1=madj_sb[:],
                                    op=A.add)
            nc.scalar.activation(msel[:], msel[:], AF.Square, accum_out=lm)
        else:
            nc.vector.tensor_copy(lm, nvec)

        if "c" in sub:
            # covs loss over the full augmented grid; rnrdw folded into one
            # outer-product factor (saves a full-grid pass)
            s1s = tmp.tile([SLOTS, DA], f32)
            nc.vector.tensor_scalar(s1s[:], s1, rnrdw, None, op0=A.mult)
            s1_b = s1s[:].rearrange("p (c o) -> p c o",
                                    o=1).to_broadcast((SLOTS, DA, DA))
            s1_a = s1.rearrange("p (o a) -> p o a",
                                o=1).to_broadcast((SLOTS, DA, DA))
            og3 = og[:].rearrange("p (c a) -> p c a", a=DA)
            nc.vector.tensor_tensor(out=og3, in0=s1_b, in1=s1_a, op=A.mult)
            nc.vector.tensor_scalar(qg[:], s2[:], rdw, None, op0=A.mult)
            nc.vector.tensor_tensor(out=qg[:], in0=qg[:], in1=og[:],
                                    op=A.subtract)
            nc.vector.tensor_tensor(out=qg[:], in0=qg[:], in1=tf_sb[:],
                                    op=A.subtract)
            nc.scalar.activation(qg[:], qg[:], AF.Square, accum_out=lc)
        else:
            nc.vector.tensor_copy(lc, nvec)

        if "l" in sub:
            # filling loss: lf = (n/N - ft)^2
            fv = tmp.tile([SLOTS, 1], f32)
            nc.vector.tensor_scalar(fv[:], nvec, 1.0 / N, ftneg,
                                    op0=A.mult, op1=A.add)
            nc.vector.tensor_tensor(out=lf, in0=fv[:], in1=fv[:], op=A.mult)
        else:
            nc.vector.tensor_copy(lf, nvec)
        nc.vector.tensor_copy(red[:, 1:2], lm)
        nc.vector.tensor_copy(red[:, 2:3], lc)
        nc.vector.tensor_copy(red[:, 3:4], nvec)
        nc.sync.dma_start(out[:], red[:])

    if "f" not in phases:
        # bench variants: keep the last phase's result live via a tiny dump
        keep = tmp.tile([SLOTS, 4], f32)
        src = s2 if "s" in phases else (tg if "t" in phases else gall)
        nc.scalar.activation(keep[:], src[0:SLOTS, 0:4], AF.Copy)
        nc.sync.dma_start(out[:], keep[:])


def _get_program(name, rounds=1, **kw):
    key = (name, rounds, tuple(sorted(kw.items())))
    if key not in _programs:
        build = _build_stage1 if name == "s1" else _build_stage2
        _programs[key] = build(rounds=rounds, **kw)
    return _programs[key]


def _bench(name, in_maps, rounds, repeats=5, **kw):
    """Wall time of one NEFF containing `rounds` copies of the stage body.

    Difference two rounds-counts to get the marginal per-round device time
    (fixed dispatch/RPC/DMA-in overhead cancels).
    """
    import time
    import jax
    from jax.sharding import Mesh, PartitionSpec
    from jax.experimental.shard_map import shard_map
    import concourse.mybir as mybir
    from concourse.bass2jax import _bass_exec_p, install_neuronx_cc_hook
    from concourse.bass2jax import partition_id_tensor

    nc = _get_program(name, rounds=rounds, **kw)
    install_neuronx_cc_hook()
    pname = nc.partition_id_tensor.name if nc.partition_id_tensor else None
    in_names, out_names, out_avals, zero_outs = [], [], [], []
    for alloc in nc.m.functions[0].allocations:
        if not isinstance(alloc, mybir.MemoryLocationSet):
            continue
        nm = alloc.memorylocations[0].name
        if alloc.kind == "ExternalInput":
            if nm != pname:
                in_names.append(nm)
        elif alloc.kind == "ExternalOutput":
            out_names.append(nm)
            shape = tuple(alloc.tensor_shape)
            dtype = mybir.dt.np(alloc.dtype)
            out_avals.append(jax.core.ShapedArray(shape, dtype))
            zero_outs.append(np.zeros(shape, dtype))
    n_params = len(in_names)
    all_in = in_names + out_names

    if pname is not None:
        all_in = all_in + [pname]

    def _body(*args):
        operands = list(args)
        if pname is not None:
            operands.append(partition_id_tensor())
        outs = _bass_exec_p.bind(
            *operands,
            out_avals=tuple(out_avals),
            in_names=tuple(all_in),
            out_names=tuple(out_names),
            lowering_input_output_aliases=(),
            sim_require_finite=True,
            sim_require_nnan=True,
            nc=nc,
        )
        return tuple(outs)

    devices = jax.devices()[:NCORES]
    mesh = Mesh(np.asarray(devices), ("core",))
    specs = (PartitionSpec("core"),) * (n_params + len(out_names))
    fn = jax.jit(shard_map(_body, mesh=mesh, in_specs=specs,
                           out_specs=(PartitionSpec("core"),) * len(out_names),
                           check_rep=False))
    concat = [np.concatenate([np.asarray(m[nm]) for m in in_maps], axis=0)
              for nm in in_names]
    concat += [np.zeros((NCORES * z.shape[0], *z.shape[1:]), z.dtype)
               for z in zero_outs]
    concat = [jax.device_put(a) for a in concat]
    jax.block_until_ready(fn(*concat))  # compile + warm
    best = float("inf")
    for _ in range(repeats):
        t0 = time.perf_counter()
        jax.block_until_ready(fn(*concat))
        best = min(best, time.perf_counter() - t0)
    return best


def _bench_pipelined(name, in_maps, rounds=104, ncalls=30, **kw):
    """Per-round upper bound: many async executions of a big-rounds NEFF.

    Dispatch cost amortizes across `ncalls` in-flight executions; with
    `rounds` large the device time dominates.  Returns seconds per round.
    """
    import time
    import jax
    from jax.sharding import Mesh, PartitionSpec
    from jax.experimental.shard_map import shard_map
    import concourse.mybir as mybir
    from concourse.bass2jax import (_bass_exec_p, install_neuronx_cc_hook,
                                    partition_id_tensor)

    nc = _get_program(name, rounds=rounds, **kw)
    install_neuronx_cc_hook()
    pname = nc.partition_id_tensor.name if nc.partition_id_tensor else None
    in_names, out_names, out_avals, zero_outs = [], [], [], []
    for alloc in nc.m.functions[0].allocations:
        if not isinstance(alloc, mybir.MemoryLocationSet):
            continue
        nm = alloc.memorylocations[0].name
        if alloc.kind == "ExternalInput":
            if nm != pname:
                in_names.append(nm)
        elif alloc.kind == "ExternalOutput":
            out_names.append(nm)
            shape = tuple(alloc.tensor_shape)
            dtype = mybir.dt.np(alloc.dtype)
            out_avals.append(jax.core.ShapedArray(shape, dtype))
            zero_outs.append(np.zeros(shape, dtype))
    n_params = len(in_names)
    all_in = in_names + out_names
    if pname is not None:
        all_in = all_in + [pname]

    def _body(*args):
        operands = list(args)
        if pname is not None:
            operands.append(partition_id_tensor())
        return tuple(_bass_exec_p.bind(
            *operands,
            out_avals=tuple(out_avals),
            in_names=tuple(all_in),
            out_names=tuple(out_names),
            lowering_input_output_aliases=(),
            sim_require_finite=True,
            sim_require_nnan=True,
            nc=nc,
        ))

    devices = jax.devices()[:NCORES]
    mesh = Mesh(np.asarray(devices), ("core",))
    specs = (PartitionSpec("core"),) * (n_params + len(out_names))
    fn = jax.jit(shard_map(_body, mesh=mesh, in_specs=specs,
                           out_specs=(PartitionSpec("core"),) * len(out_names),
                           check_rep=False))
    concat = [np.concatenate([np.asarray(m[nm]) for m in in_maps], axis=0)
              for nm in in_names]
    concat += [np.zeros((NCORES * z.shape[0], *z.shape[1:]), z.dtype)
               for z in zero_outs]
    concat = [jax.device_put(a) for a in concat]
    jax.block_until_ready(fn(*concat))  # compile + warm
    best = float("inf")
    for _ in range(3):
        outs = []
        t0 = time.perf_counter()
        for _ in range(ncalls):
            outs.append(fn(*concat))
        jax.block_until_ready(outs)
        best = min(best, (time.perf_counter() - t0) / (ncalls * rounds))
    return best


# --------------------------------------------------------------------------
# host planning
# --------------------------------------------------------------------------

def _plan(pred):
    """Greedy bin-pack clusters onto cores, balancing 128-row tile counts."""
    counts = np.bincount(pred, minlength=K)
    tiles = -(-counts // PT)  # ceil; 0 for empty clusters
    order = np.argsort(-tiles, kind="stable")
    load = np.zeros(NCORES, np.int64)
    nslots = np.zeros(NCORES, np.int64)
    slots = [[] for _ in range(NCORES)]
    for k in order:
        cands = [c for c in range(NCORES) if nslots[c] < SLOTS]
        c = min(cands, key=lambda c: (load[c], c))
        slots[c].append(int(k))
        load[c] += int(tiles[k])
        nslots[c] += 1
    ok = load.max() <= T2 and tiles.max() <= T2
    return counts, tiles, slots, ok


def _stage2_inputs(slots_c, counts, order, starts, xaug16z, ct, mt, cents, ft):
    """Build one core's stage-2 tensors."""
    idx = np.full((T2, PT), N, dtype=np.int64)  # N -> zero row
    A = np.zeros((T2, SLOTS), BF16)
    t = 0
    for j, k in enumerate(slots_c):
        rows = order[starts[k]:starts[k + 1]]
        for w in range(0, len(rows), PT):
            chunk = rows[w:w + PT]
            idx[t, :len(chunk)] = chunk
            A[t, j] = 1.0
            t += 1
    g = xaug16z[idx.reshape(-1)].reshape(T2, PT, DA)       # [t, p, c]
    xg = np.ascontiguousarray(g.transpose(1, 0, 2)).reshape(PT, T2 * DA)

    tf = np.zeros((SLOTS, DA, DA), np.float32)
    tf[:, :D, :D] = ct[slots_c].transpose(0, 2, 1)          # tf[j,c,a]=T[a,c]

    n = counts[slots_c].astype(np.float64)
    rn = 1.0 / np.maximum(n, 1.0)
    rdw = (n > 1) / np.maximum(n - 1.0, 1.0)
    z = (n > 0).astype(np.float64)
    madj = ((1.0 - z)[:, None] * cents[slots_c] - mt[slots_c])
    hscal = np.stack([rn, rdw, rn * rdw, -ft[slots_c]], axis=1)
    return {
        "xg": xg,
        "sel": A,
        "tfull": tf.reshape(SLOTS, GRID),
        "madj": madj.astype(np.float32),
        "hscal": hscal.astype(np.float32),
    }


# --------------------------------------------------------------------------
# entry points
# --------------------------------------------------------------------------

def _kernel_host(x, cluster_centers, filling_target, means_target, covs_target,
                 pred=None):
    """Numpy fallback (also used to finish from a device-computed pred)."""
    c2 = (cluster_centers * cluster_centers).sum(axis=1)
    if pred is None:
        pred = np.empty(N, dtype=np.int64)
        for i in range(0, N, 8192):
            sc = x[i:i + 8192] @ cluster_centers.T - 0.5 * c2
            pred[i:i + 8192] = sc.argmax(axis=1)
    counts = np.bincount(pred, minlength=K).astype(np.float32)
    loss_fil = np.mean((counts / np.float32(N) - filling_target) ** 2,
                       dtype=np.float32)
    sums = np.zeros((K, D), np.float32)
    np.add.at(sums, pred, x)
    safe = np.maximum(counts, 1.0)
    means = np.where(counts[:, None] > 0, sums / safe[:, None], cluster_centers)
    order = np.argsort(pred, kind="stable")
    xs = x[order]
    starts = np.zeros(K + 1, np.int64)
    starts[1:] = np.cumsum(counts).astype(np.int64)
    S = np.zeros((K, D, D), np.float32)
    for k in range(K):
        a, b = starts[k], starts[k + 1]
        if b > a:
            S[k] = xs[a:b].T @ xs[a:b]
    denom = np.maximum(counts - 1.0, 1.0)
    covs = (S - counts[:, None, None] * means[:, :, None] * means[:, None, :]) \
        / denom[:, None, None]
    covs = np.where(counts[:, None, None] > 1, covs, 0.0).astype(np.float32)
    loss_stat = np.mean((means - means_target) ** 2, dtype=np.float32) + \
        np.mean((covs - covs_target) ** 2, dtype=np.float32)
    return np.float32(loss_fil + loss_stat)


def _kernel_trn(x, cluster_centers, filling_target, means_target, covs_target):
    from concourse.bass_utils import run_bass_kernel_spmd

    core_ids = list(range(NCORES))

    # ---- stage 1 ----
    c2 = (cluster_centers * cluster_centers).sum(axis=1)
    xaugT = np.empty((DA, N), BF16)
    xaugT[:D] = x.T.astype(BF16)
    xaugT[D] = 1.0
    caug = np.empty((DA, K), BF16)
    caug[:D] = cluster_centers.T.astype(BF16)
    caug[D] = (-0.5 * c2).astype(BF16)
    in1 = [{"xT": np.ascontiguousarray(xaugT[:, c * NPC:(c + 1) * NPC]),
            "caug": caug} for c in core_ids]
    LAST_INPUTS["s1"] = in1
    r1 = run_bass_kernel_spmd(_get_program("s1"), in1, core_ids)
    pred = np.empty(N, np.int64)
    for c in core_ids:
        pc = r1.results[c]["pred"].reshape(PT, NT1, 8)[:, :, 0]   # [PT, NT1]
        pred[c * NPC:(c + 1) * NPC] = pc.T.reshape(NPC).astype(np.int64)

    # ---- host planning ----
    counts, tiles, slots, ok = _plan(pred)
    if not ok:
        return _kernel_host(x, cluster_centers, filling_target, means_target,
                            covs_target, pred=pred)
    order = np.argsort(pred, kind="stable")
    starts = np.zeros(K + 1, np.int64)
    starts[1:] = np.cumsum(counts)
    xaug16z = np.zeros((N + 1, DA), BF16)
    xaug16z[:N, :D] = x.astype(BF16)
    xaug16z[:N, D] = 1.0

    in2 = [_stage2_inputs(slots[c], counts, order, starts, xaug16z,
                          covs_target, means_target, cluster_centers,
                          filling_target) for c in core_ids]
    LAST_INPUTS["s2"] = in2
    r2 = run_bass_kernel_spmd(_get_program("s2"), in2, core_ids)
    lf = lm = lc = 0.0
    for c in core_ids:
        p = r2.results[c]["partial"].sum(axis=0, dtype=np.float64)
        lf += float(p[0]); lm += float(p[1]); lc += float(p[2])
    loss = lf / K + lm / (K * D) + lc / (K * D * D)
    return np.float32(loss)


def kernel(x, cluster_centers, filling_target, means_target, covs_target):
    x = np.asarray(x, dtype=np.float32)
    cluster_centers = np.asarray(cluster_centers, dtype=np.float32)
    filling_target = np.asarray(filling_target, dtype=np.float32)
    means_target = np.asarray(means_target, dtype=np.float32)
    covs_target = np.asarray(covs_target, dtype=np.float32)
    try:
        return _kernel_trn(x, cluster_centers, filling_target, means_target,
                           covs_target)
    except Exception:
        import traceback
        traceback.print_exc()
        return _kernel_host(x, cluster_centers, filling_target, means_target,
                            covs_target)
